# revision 12
# baseline (speedup 1.0000x reference)
"""ConvDeepSet Trainium2 kernel.

Math (per batch b):
    agg[m, c] = sum_n ycat[n, c] * exp(-alpha_c * (x[n] - t[m])^2)
    density   = agg[:, 0];  conv_j = agg[:, j] / (density + eps)
    out[m, o] = w[o, 0] * density[m] + sum_j w[o, j] * conv_j[m] + bias[o]

Strategy:
  - Data parallel: 16 batches over 8 cores (2 per core), no collectives.
  - Distance matrix d[n, m] built ON the tensor engine as a K=7 rank
    decomposition  d = x^2 - 2xt + t^2, with every operand split hi/lo in
    fp16 so the (exactly accumulated) fp16 products reproduce d to ~1e-5
    absolute (fp16 matmuls run 4x faster than fp32 on PE).
  - exp on the scalar engine (only engine with transcendentals), writing
    fp16 E tiles; channel reduction ycat^T @ E on PE in fp16 (exact
    product accumulation into fp32 PSUM).
  - 1/(density+eps) as exp(-ln(density+eps)) on ACT: ln and exp live in
    the same activation-table set (reciprocal does not).
  - Channels grouped by identical length scale: one exp pass per distinct
    alpha (the shipped model has all 8 sigmas equal -> a single pass).
"""

import sys

sys.path.insert(0, "/opt/trn_rl_repo")

import numpy as np

B, N, M, CIN, C, COUT = 16, 1024, 1024, 7, 8, 16
NCORES = 8
BPC = B // NCORES
P = 128
NT = N // P          # 8 n-tiles per batch
MTILE = 512          # psum-bank-limited matmul width
MT = M // MTILE      # 2 m-tiles
EPS = 1e-8

TRACE = False
DEBUG = False
LAST_RESULTS = None

_cache = {}


def _build(groups):
    """groups: tuple of (c0, csz, alpha) with contiguous channel spans."""
    from contextlib import ExitStack

    import concourse.mybir as mybir
    import concourse.tile as tile
    from concourse import bacc

    F32 = mybir.dt.float32
    F16 = mybir.dt.float16
    AF = mybir.ActivationFunctionType

    nc = bacc.Bacc("TRN2")
    x_d = nc.dram_tensor("x", [BPC, N], F32, kind="ExternalInput").ap()
    t_d = nc.dram_tensor("t", [BPC, M], F32, kind="ExternalInput").ap()
    ycat_d = nc.dram_tensor("ycat", [BPC, N, C], F32, kind="ExternalInput").ap()
    wt_d = nc.dram_tensor("wt", [C, COUT], F32, kind="ExternalInput").ap()
    bcol_d = nc.dram_tensor("bcol", [COUT, 1], F32, kind="ExternalInput").ap()
    out_d = nc.dram_tensor("out", [BPC, COUT, M], F32, kind="ExternalOutput").ap()
    if DEBUG:
        dbg_agg_d = nc.dram_tensor("dbg_agg", [BPC, MT, C, MTILE], F32, kind="ExternalOutput").ap()
        dbg_rec_d = nc.dram_tensor("dbg_rec", [BPC, MT, MTILE], F32, kind="ExternalOutput").ap()
        dbg_feat_d = nc.dram_tensor("dbg_feat", [BPC, MT, C, MTILE], F32, kind="ExternalOutput").ap()
        dbg_d_d = nc.dram_tensor("dbg_d", [P, M], F32, kind="ExternalOutput").ap()
        dbg_e_d = nc.dram_tensor("dbg_e", [P, M], F32, kind="ExternalOutput").ap()

    with tile.TileContext(nc) as tc, ExitStack() as ctx:
        consts = ctx.enter_context(tc.tile_pool(name="consts", bufs=1))
        stage = ctx.enter_context(tc.tile_pool(name="stage", bufs=2))
        rows = ctx.enter_context(tc.tile_pool(name="rows", bufs=2))
        ypool = ctx.enter_context(tc.tile_pool(name="ypool", bufs=2))
        epool = ctx.enter_context(tc.tile_pool(name="epool", bufs=4))
        episb = ctx.enter_context(tc.tile_pool(name="episb", bufs=2))
        dps = ctx.enter_context(tc.tile_pool(name="dps", bufs=2, space="PSUM"))
        aggps = ctx.enter_context(tc.tile_pool(name="aggps", bufs=4, space="PSUM"))

        # ---- constants ----
        wt_sb = consts.tile([C, COUT], F32)
        nc.sync.dma_start(wt_sb[:], wt_d[:])
        wth8 = consts.tile([C, COUT], F16)     # row 0 zeroed (density handled apart)
        nc.vector.tensor_copy(wth8[:], wt_sb[:])
        nc.vector.memset(wth8[0:1, :], 0.0)
        wth0 = consts.tile([1, COUT], F16)
        nc.vector.tensor_copy(wth0[:], wt_sb[0:1, :])
        bcol_sb = consts.tile([COUT, 1], F32)
        nc.sync.dma_start(bcol_sb[:], bcol_d[:])
        ones8f = consts.tile([1, C], F32)
        nc.vector.memset(ones8f[:], 1.0)
        eps_sb = consts.tile([MT, 1], F32)
        nc.vector.memset(eps_sb[:], EPS)
        onesrow = consts.tile([1, M], F16)
        nc.vector.memset(onesrow[:], 1.0)

        for b_i in range(BPC):
            # ---- staging: derive hi/lo rows in [NT, P] layout ----
            xs = stage.tile([NT, P], F32, tag="xs")
            ts = stage.tile([NT, P], F32, tag="ts")
            nc.sync.dma_start(xs[:], x_d[b_i].rearrange("(e p) -> e p", p=P))
            nc.sync.dma_start(ts[:], t_d[b_i].rearrange("(e p) -> e p", p=P))
            x2 = stage.tile([NT, P], F32, tag="x2")
            t2 = stage.tile([NT, P], F32, tag="t2")
            nc.vector.tensor_mul(x2[:], xs[:], xs[:])
            nc.vector.tensor_mul(t2[:], ts[:], ts[:])

            xh = stage.tile([NT, P], F16, tag="xh")
            xl = stage.tile([NT, P], F16, tag="xl")
            x2h = stage.tile([NT, P], F16, tag="x2h")
            x2l = stage.tile([NT, P], F16, tag="x2l")
            th = stage.tile([NT, P], F16, tag="th")
            tl = stage.tile([NT, P], F16, tag="tl")
            t2h = stage.tile([NT, P], F16, tag="t2h")
            t2l = stage.tile([NT, P], F16, tag="t2l")
            nc.vector.tensor_copy(xh[:], xs[:])
            nc.vector.tensor_sub(xl[:], xs[:], xh[:])
            nc.vector.tensor_copy(x2h[:], x2[:])
            nc.vector.tensor_sub(x2l[:], x2[:], x2h[:])
            nc.vector.tensor_copy(th[:], ts[:])
            nc.vector.tensor_sub(tl[:], ts[:], th[:])
            nc.vector.tensor_copy(t2h[:], t2[:])
            nc.vector.tensor_sub(t2l[:], t2[:], t2h[:])
            m2xh = stage.tile([NT, P], F16, tag="m2xh")
            m2xl = stage.tile([NT, P], F16, tag="m2xl")
            nc.vector.tensor_scalar_mul(m2xh[:], xh[:], -2.0)
            nc.vector.tensor_scalar_mul(m2xl[:], xl[:], -2.0)

            # ---- repack to matmul row layout ----
            # lhs7 rows: x2h x2l 1 1 m2xh m2xh m2xl   (K on partitions, n free)
            # rhs7 rows: 1 1 t2h t2l th tl th         (K on partitions, m free)
            lhs7 = rows.tile([7, N], F16, tag="lhs7")
            rhs7 = rows.tile([7, M], F16, tag="rhs7")
            for dst, src in ((lhs7[2:3, :], onesrow[:]), (lhs7[3:4, :], onesrow[:]),
                             (rhs7[0:1, :], onesrow[:]), (rhs7[1:2, :], onesrow[:]),
                             (lhs7[0:1, :], x2h[:]), (lhs7[1:2, :], x2l[:]),
                             (lhs7[4:5, :], m2xh[:]), (lhs7[5:6, :], m2xh[:]),
                             (lhs7[6:7, :], m2xl[:]),
                             (rhs7[2:3, :], t2h[:]), (rhs7[3:4, :], t2l[:]),
                             (rhs7[4:5, :], th[:]), (rhs7[5:6, :], tl[:]),
                             (rhs7[6:7, :], th[:])):
                nc.sync.dma_start(dst, src)

            # ---- ycat -> [P, NT*C] fp16 ----
            ycf = ypool.tile([P, NT * C], F32, tag="ycf")
            nc.sync.dma_start(
                ycf[:].rearrange("p (nt c) -> p nt c", nt=NT),
                ycat_d[b_i].rearrange("(nt p) c -> p nt c", p=P),
            )
            ych = ypool.tile([P, NT * C], F16, tag="ych")
            nc.vector.tensor_copy(ych[:], ycf[:])

            # ---- main loop: per alpha-group, accumulate agg over n-tiles ----
            a_sbs = []
            if len(groups) > 1:
                a_sbs = [episb.tile([C, MTILE], F32, tag="aggsb", bufs=2 * MT,
                                    name=f"asb_{b_i}_{mi}") for mi in range(MT)]
            for (c0, csz, alpha) in groups:
                aggs_g = [aggps.tile([csz, MTILE], F32, tag="agg",
                                     name=f"agg_{b_i}_{c0}_{mi}") for mi in range(MT)]
                for ni in range(NT):
                    d_ps = dps.tile([P, M], F32, tag="d")
                    lh = lhs7[:, ni * P:(ni + 1) * P]
                    for mi in range(MT):
                        nc.tensor.matmul(d_ps[:, mi * MTILE:(mi + 1) * MTILE],
                                         lhsT=lh,
                                         rhs=rhs7[:, mi * MTILE:(mi + 1) * MTILE],
                                         start=True, stop=True)
                    e_sb = epool.tile([P, M], F16, tag="E")
                    nc.scalar.activation(e_sb[:], d_ps[:], AF.Exp, scale=-float(alpha))
                    if DEBUG and b_i == 0 and ni == 5:
                        dbg1 = episb.tile([P, M], F32, tag="dbg1")
                        nc.vector.tensor_copy(dbg1[:], d_ps[:])
                        nc.sync.dma_start(dbg_d_d[:], dbg1[:])
                        dbg2 = episb.tile([P, M], F32, tag="dbg2")
                        nc.vector.tensor_copy(dbg2[:], e_sb[:])
                        nc.sync.dma_start(dbg_e_d[:], dbg2[:])
                    for mi in range(MT):
                        nc.tensor.matmul(
                            aggs_g[mi][:],
                            lhsT=ych[:, ni * C + c0: ni * C + c0 + csz],
                            rhs=e_sb[:, mi * MTILE:(mi + 1) * MTILE],
                            start=(ni == 0), stop=(ni == NT - 1),
                        )
                for mi in range(MT):
                    piece = episb.tile([csz, MTILE], F32, tag="aggsb", bufs=2 * MT,
                                       name=f"piece_{b_i}_{c0}_{mi}")
                    nc.vector.tensor_copy(piece[:], aggs_g[mi][:])
                    if len(groups) == 1:
                        a_sbs.append(piece)
                    else:
                        nc.sync.dma_start(a_sbs[mi][c0:c0 + csz, :], piece[:])

            # ---- epilogue (per batch) ----
            dens2 = episb.tile([MT, MTILE], F32, tag="dens2")
            for mi in range(MT):
                nc.sync.dma_start(dens2[mi:mi + 1, :], a_sbs[mi][0:1, :])
            dens2e = episb.tile([MT, MTILE], F32, tag="dens2e")
            nc.vector.tensor_scalar_add(dens2e[:], dens2[:], EPS)
            ln2 = episb.tile([MT, MTILE], F32, tag="ln2")
            nc.scalar.activation(ln2[:], dens2e[:], AF.Ln)
            rec2 = episb.tile([MT, MTILE], F32, tag="rec2")
            nc.scalar.activation(rec2[:], ln2[:], AF.Exp, scale=-1.0)

            for mi in range(MT):
                a_sb = a_sbs[mi]
                rech = episb.tile([1, MTILE], F32, tag="rech")
                nc.sync.dma_start(rech[:], rec2[mi:mi + 1, :])
                if DEBUG:
                    nc.sync.dma_start(dbg_agg_d[b_i, mi], a_sb[:])
                    dbgr = episb.tile([1, MTILE], F32, tag="dbgr")
                    nc.vector.tensor_copy(dbgr[:], rech[:])
                    nc.sync.dma_start(dbg_rec_d[b_i, mi:mi+1], dbgr[:])
                rb_ps = aggps.tile([C, MTILE], F32, tag="agg", name=f"rb_{b_i}_{mi}")
                nc.tensor.matmul(rb_ps[:], lhsT=ones8f[:],
                                 rhs=rech[:], start=True, stop=True)
                feats8 = episb.tile([C, MTILE], F16, tag="feats")
                nc.vector.tensor_mul(feats8[:], a_sb[:], rb_ps[:])
                if DEBUG:
                    dbgf = episb.tile([C, MTILE], F32, tag="dbgf")
                    nc.vector.tensor_copy(dbgf[:], feats8[:])
                    nc.sync.dma_start(dbg_feat_d[b_i, mi], dbgf[:])
                densh = episb.tile([1, MTILE], F16, tag="densh")
                nc.vector.tensor_copy(densh[:], a_sb[0:1, :])
                o_ps = aggps.tile([COUT, MTILE], F32, tag="agg", name=f"o_{b_i}_{mi}")
                nc.tensor.matmul(o_ps[:], lhsT=wth8[:], rhs=feats8[:],
                                 start=True, stop=False)
                nc.tensor.matmul(o_ps[:], lhsT=wth0[:], rhs=densh[:],
                                 start=False, stop=True)
                o_sb = episb.tile([COUT, MTILE], F32, tag="osb")
                nc.vector.tensor_scalar(o_sb[:], o_ps[:], bcol_sb[:], None,
                                        op0=mybir.AluOpType.add)
                nc.sync.dma_start(out_d[b_i][:, mi * MTILE:(mi + 1) * MTILE], o_sb[:])

    nc.compile()
    return nc


def _prep(x, y, t, sigma, w, b):
    x = np.ascontiguousarray(np.asarray(x, np.float32).reshape(B, N))
    t = np.ascontiguousarray(np.asarray(t, np.float32).reshape(B, M))
    y = np.asarray(y, np.float32)
    sigma = np.asarray(sigma, np.float32)
    w = np.asarray(w, np.float32)
    b = np.asarray(b, np.float32)

    scales = np.exp(sigma.astype(np.float64))
    alphas = (0.5 / scales ** 2).astype(np.float64)

    # group channels by identical alpha; channel 0 (density) leads its group
    perm = [0]
    groups = []
    used = np.zeros(C, bool)
    order = [0] + [c for c in range(1, C)]
    for c in order:
        if used[c]:
            continue
        members = [cc for cc in range(C) if not used[cc] and alphas[cc] == alphas[c]]
        if c == 0:
            members = [0] + [m for m in members if m != 0]
        for m_ in members:
            used[m_] = True
        groups.append(members)
    perm = [c for g in groups for c in g]
    # groups as (c0, csz, alpha) over permuted channel axis
    gspec = []
    c0 = 0
    for g in groups:
        gspec.append((c0, len(g), float(alphas[g[0]])))
        c0 += len(g)
    gspec = tuple(gspec)

    ycat = np.concatenate([np.ones((B, N, 1), np.float32), y], axis=2)
    ycat = np.ascontiguousarray(ycat[:, :, perm])
    wt = np.ascontiguousarray(w[:, perm].T)          # [C, COUT]
    bcol = np.ascontiguousarray(b.reshape(COUT, 1))
    return x, t, ycat, wt, bcol, gspec


def kernel(x, y, t, sigma, w, b):
    global LAST_RESULTS
    from concourse.bass_utils import run_bass_kernel_spmd

    xf, tf, ycat, wt, bcol, gspec = _prep(x, y, t, sigma, w, b)
    if (gspec, DEBUG) not in _cache:
        _cache[(gspec, DEBUG)] = _build(gspec)
    nc = _cache[(gspec, DEBUG)]

    in_maps = []
    for core in range(NCORES):
        sl = slice(core * BPC, (core + 1) * BPC)
        in_maps.append({
            "x": xf[sl], "t": tf[sl], "ycat": ycat[sl],
            "wt": wt, "bcol": bcol,
        })
    res = run_bass_kernel_spmd(nc, in_maps, list(range(NCORES)), trace=TRACE)
    LAST_RESULTS = res
    out = np.concatenate([res.results[i]["out"] for i in range(NCORES)], axis=0)
    return np.ascontiguousarray(out.transpose(0, 2, 1)).astype(np.float32)
